# revision 3
# baseline (speedup 1.0000x reference)
"""Multi-head attention Trainium2 kernel (8 NeuronCores).

Problem: x[2,2048,1024] -> MHA(16 heads, d=64) -> out[2,2048,1024], fp32.

Sharding: 2-way data parallel on batch x 4-way tensor parallel on heads.
Core c handles batch c//4 and heads 4*(c%4) .. 4*(c%4)+3 (a 256-wide slice
of the Wq/Wk/Wv columns and Wo rows). Each core returns a partial output
[2048,1024]; the host sums the 4 TP partials per batch and adds the bias
terms (bo, and bv@Wo which is separable because softmax rows sum to 1;
bk drops out of softmax entirely since (q+bq)@bk is constant along keys).

On-core dataflow (all-bf16 operands, fp32 PSUM accumulation):
  xt = x[b].T (host-transposed, bf16)  [1024, 2048], DMA'd in 8 ec chunks
  V natural [2048, 256] via xt-stationary matmuls, ec-outer across 4 psum
    tiles so the chains consume each xt chunk as its DMA lands
  Q^T/K^T = W^T stationary over xt     [256, 2048]
  S^T[k,q] = K^T(d,k).T @ Q^T(d,q)     2 heads, d=64 each
  P = exp(S^T / 32)                    ScalarE only (1024-wide ACTs), bf16
  O'^T[d+1,q] = [V|ones].T @ P         ones column gives softmax denominators
  O^T = O'[0:64] * (1/denom)           DVE direct from PSUM, approx-recip
  out = O^T.T @ Wo_g                   [2048, 1024] partial, DMA'd out

The attention loop is software-pipelined so ScalarE (exp) stays saturated:
per kc the PE queue gets S(kc,h0) S(kc,h1) PV(kc-1,h1) PV(kc,h0); PV(kc,h1)
is deferred one iteration so it never head-of-line-blocks the next kc's S.
"""

import numpy as np

B = 2
N = 2048
E = 1024
HEADS = 16
D = 64
P = 128
NCORES = 8
GROUPS = 4            # TP groups
DG = E // GROUPS      # 256 cols per core
ECH = E // P          # 8 contraction chunks
NCH = N // P          # 16 sequence chunks
QS = 1024             # q span for softmax tiles
QB = 512              # matmul moving free dim

_CACHE = {}


def _build():
    import sys
    if "/opt/trn_rl_repo" not in sys.path:
        sys.path.insert(0, "/opt/trn_rl_repo")
    import concourse.tile as tile
    from concourse import bacc, mybir
    from concourse.bass import ts

    F32 = mybir.dt.float32
    BF16 = mybir.dt.bfloat16
    Exp = mybir.ActivationFunctionType.Exp

    nc = bacc.Bacc("TRN2", target_bir_lowering=False, debug=False, num_devices=NCORES)

    xt = nc.dram_tensor("xt", [E, N], BF16, kind="ExternalInput").ap()
    wq = nc.dram_tensor("wq", [E, DG], BF16, kind="ExternalInput").ap()
    wk = nc.dram_tensor("wk", [E, DG], BF16, kind="ExternalInput").ap()
    wv = nc.dram_tensor("wv", [E, DG], BF16, kind="ExternalInput").ap()
    wo = nc.dram_tensor("wo", [DG, E], BF16, kind="ExternalInput").ap()
    bq2 = nc.dram_tensor("bq2", [P, 2], F32, kind="ExternalInput").ap()
    out = nc.dram_tensor("out", [N, E], F32, kind="ExternalOutput").ap()

    with tile.TileContext(nc) as tc:
        with tc.tile_pool(name="persist", bufs=1) as pers, \
             tc.tile_pool(name="pexp", bufs=12) as pexp_pool, \
             tc.tile_pool(name="small", bufs=2) as small, \
             tc.tile_pool(name="ostage", bufs=6) as ostage, \
             tc.tile_pool(name="ppmain", bufs=1, space="PSUM") as ppm, \
             tc.tile_pool(name="ppoacc", bufs=1, space="PSUM") as ppo:
            wq_sb = pers.tile([P, ECH, DG], BF16, tag="wq")
            wk_sb = pers.tile([P, ECH, DG], BF16, tag="wk")
            wv_sb = pers.tile([P, ECH, DG], BF16, tag="wv")
            wo_sb = pers.tile([P, 2, E], BF16, tag="wo")
            bq_sb = pers.tile([P, 2], F32, tag="bq")
            qT_p = [pers.tile([P, N], BF16, tag=f"qT{i}", name=f"qT{i}") for i in range(2)]
            kT_p = [pers.tile([P, N], BF16, tag=f"kT{i}", name=f"kT{i}") for i in range(2)]
            v_sb = pers.tile([P, NCH, GROUPS, 66], BF16, tag="v")
            oT_p = [pers.tile([P, N], BF16, tag=f"oT{i}", name=f"oT{i}") for i in range(2)]
            xt_sb = [pers.tile([P, N], BF16, tag=f"xt{ec}", name=f"xt{ec}")
                     for ec in range(ECH)]

            def psum_by_idx(i, name):
                pool, tag = [(ppm, "A"), (ppm, "B"), (ppo, "O0"), (ppo, "O1")][i % 4]
                return pool.tile([P, QS], F32, tag=tag, name=name)

            def qk_chain(pair, w_sb, dst, bias, qb):
                ps = psum_by_idx(qb, f"qkps{pair}{qb}")
                psl = ps[:, :QB]
                for ec in range(ECH):
                    nc.tensor.matmul(
                        psl,
                        w_sb[:, ec, ts(pair, P)],
                        xt_sb[ec][:, ts(qb, QB)],
                        start=(ec == 0), stop=(ec == ECH - 1),
                    )
                if bias:
                    nc.vector.tensor_add(
                        dst[:, ts(qb, QB)], psl,
                        bq_sb[:, pair, None].to_broadcast((P, QB)),
                    )
                else:
                    nc.vector.tensor_copy(dst[:, ts(qb, QB)], psl)

            def wo_chain(ncx, fb):
                ps = psum_by_idx(ncx * 2 + fb, f"wops{ncx}{fb}")
                psl = ps[:, :QB]
                for dc in range(2):
                    nc.tensor.matmul(
                        psl,
                        oT_p[dc][:, ts(ncx, P)],
                        wo_sb[:, dc, ts(fb, QB)],
                        start=(dc == 0), stop=(dc == 1),
                    )
                ot = ostage.tile([P, QB], F32, tag="ot", name="ot")
                nc.vector.tensor_copy(ot, psl)
                nc.sync.dma_start(out[ts(ncx, P), ts(fb, QB)], ot)

            def emit_v():
                # ec-outer over 4 concurrent psum tiles (4 token chunks each)
                # so each xt chunk is consumed as soon as its DMA lands
                for g in range(4):
                    pss = [psum_by_idx(i, f"vps{g}{i}") for i in range(4)]
                    for ec in range(ECH):
                        for i in range(4):
                            ncx = 4 * g + i
                            nc.tensor.matmul(
                                pss[i][:, ts(i, DG)],
                                xt_sb[ec][:, ts(ncx, P)],
                                wv_sb[:, ec, :],
                                start=(ec == 0), stop=(ec == ECH - 1),
                            )
                    for i in range(4):
                        ncx = 4 * g + i
                        nc.vector.tensor_copy(
                            v_sb[:, ncx, :, 0:64],
                            pss[i][:, ts(i, DG)].rearrange("p (h d) -> p h d", d=D),
                        )

            def emit_attn(pair, qs):
                oaccs = [ppo.tile([65, QS], F32, tag=f"O{h}", name=f"oacc{h}")
                         for h in range(2)]
                pe_prev = None
                for kc in range(NCH):
                    pss = [ppm.tile([P, QS], F32, tag="AB"[h], name=f"spsum{h}")
                           for h in range(2)]
                    for h in range(2):
                        psl = slice(D * h, D * h + D)
                        for qb in range(QS // QB):
                            nc.tensor.matmul(
                                pss[h][:, ts(qb, QB)],
                                kT_p[pair][psl, ts(kc, P)],
                                qT_p[pair][psl, qs * QS + qb * QB:qs * QS + (qb + 1) * QB],
                                start=True, stop=True,
                            )
                    if pe_prev is not None:
                        for qb in range(QS // QB):
                            nc.tensor.matmul(
                                oaccs[1][:, ts(qb, QB)],
                                v_sb[:, kc - 1, 2 * pair + 1, 0:65],
                                pe_prev[:, ts(qb, QB)],
                                start=(kc - 1 == 0), stop=(kc - 1 == NCH - 1),
                            )
                    pes = []
                    for h in range(2):
                        pe = pexp_pool.tile([P, QS], BF16, tag="pexp", name="pexp")
                        nc.scalar.activation(pe, pss[h], Exp, scale=1.0 / 32.0)
                        pes.append(pe)
                    for qb in range(QS // QB):
                        nc.tensor.matmul(
                            oaccs[0][:, ts(qb, QB)],
                            v_sb[:, kc, 2 * pair, 0:65],
                            pes[0][:, ts(qb, QB)],
                            start=(kc == 0), stop=(kc == NCH - 1),
                        )
                    pe_prev = pes[1]
                for qb in range(QS // QB):
                    nc.tensor.matmul(
                        oaccs[1][:, ts(qb, QB)],
                        v_sb[:, NCH - 1, 2 * pair + 1, 0:65],
                        pe_prev[:, ts(qb, QB)],
                        start=False, stop=True,
                    )
                for h in range(2):
                    dn = small.tile([1, QS], F32, tag=f"dn{h}", name="dn", bufs=1)
                    nc.vector.tensor_copy(dn, oaccs[h][64:65, :])
                    rc = small.tile([1, QS], F32, tag=f"rc{h}", name="rc", bufs=1)
                    nc.vector.reciprocal_approx_fast(rc, dn)
                    rbc = small.tile([64, QS], F32, tag="rbc", name="rbc")
                    nc.gpsimd.partition_broadcast(rbc, rc)
                    psl = slice(D * h, D * h + D)
                    nc.vector.tensor_mul(
                        oT_p[pair][psl, ts(qs, QS)],
                        oaccs[h][0:64, :],
                        rbc,
                    )

            nc.sync.dma_start(wv_sb, wv.rearrange("(c p) d -> p c d", p=P))
            xt_r = xt.rearrange("(c p) n -> p c n", p=P)
            for ec in range(4):
                nc.sync.dma_start(xt_sb[ec], xt_r[:, ec, :])
            nc.sync.dma_start(wk_sb, wk.rearrange("(c p) d -> p c d", p=P))
            nc.sync.dma_start(wq_sb, wq.rearrange("(c p) d -> p c d", p=P))
            for ec in range(4, ECH):
                nc.sync.dma_start(xt_sb[ec], xt_r[:, ec, :])
            nc.sync.dma_start(bq_sb, bq2)
            nc.sync.dma_start(wo_sb, wo.rearrange("(c p) f -> p c f", p=P))
            ones_f32 = pers.tile([P, 1], F32, tag="ones")
            nc.vector.memset(ones_f32, 1.0)
            nc.vector.tensor_copy(
                v_sb[:, :, :, 64:65],
                ones_f32[:, 0, None, None, None].to_broadcast((P, NCH, GROUPS, 1)),
            )
            emit_v()
            for qb in range(4):
                qk_chain(0, wk_sb, kT_p[0], False, qb)
            for qb in range(4):
                qk_chain(0, wq_sb, qT_p[0], True, qb)
            emit_attn(0, 0)
            emit_attn(0, 1)
            for qb in range(4):
                qk_chain(1, wk_sb, kT_p[1], False, qb)
            for qb in range(4):
                qk_chain(1, wq_sb, qT_p[1], True, qb)
            emit_attn(1, 0)
            emit_attn(1, 1)
            for ncx in range(NCH):
                for fb in range(2):
                    wo_chain(ncx, fb)

    nc.compile()
    return nc


def _get_nc():
    if "nc" not in _CACHE:
        _CACHE["nc"] = _build()
    return _CACHE["nc"]


def kernel(x, Wq, bq, Wk, bk, Wv, bv, Wo, bo, **run_kwargs):
    import sys
    if "/opt/trn_rl_repo" not in sys.path:
        sys.path.insert(0, "/opt/trn_rl_repo")
    import ml_dtypes
    from concourse.bass_utils import run_bass_kernel_spmd

    BF = ml_dtypes.bfloat16
    x = np.asarray(x, dtype=np.float32)
    Wq = np.asarray(Wq, dtype=np.float32)
    Wk = np.asarray(Wk, dtype=np.float32)
    Wv = np.asarray(Wv, dtype=np.float32)
    Wo = np.asarray(Wo, dtype=np.float32)
    bq = np.asarray(bq, dtype=np.float32)
    bv = np.asarray(bv, dtype=np.float32)
    bo = np.asarray(bo, dtype=np.float32)

    nc = _get_nc()

    in_maps = []
    xts = [np.ascontiguousarray(x[b].T).astype(BF) for b in range(B)]
    for c in range(NCORES):
        b, g = divmod(c, GROUPS)
        cols = slice(g * DG, (g + 1) * DG)
        in_maps.append({
            "xt": xts[b],
            "wq": np.ascontiguousarray(Wq[:, cols]).astype(BF),
            "wk": np.ascontiguousarray(Wk[:, cols]).astype(BF),
            "wv": np.ascontiguousarray(Wv[:, cols]).astype(BF),
            "wo": np.ascontiguousarray(Wo[cols, :]).astype(BF),
            "bq2": np.ascontiguousarray(bq[cols].reshape(2, P).T),
        })

    try:
        res = run_bass_kernel_spmd(nc, in_maps, core_ids=list(range(NCORES)), **run_kwargs)
    except Exception:
        # device may be wedged from a prior run; reset the accelerator once
        try:
            import ctypes
            lib = ctypes.CDLL("/opt/axon/libaxon_pjrt.so")
            lib.axon_reset.restype = ctypes.c_int
            lib.axon_reset()
        except Exception:
            pass
        res = run_bass_kernel_spmd(nc, in_maps, core_ids=list(range(NCORES)), **run_kwargs)
    if run_kwargs:
        _CACHE["last_results"] = res

    # gather: sum TP partials per batch, add separable bias terms
    bias_vec = bv @ Wo + bo  # softmax rows sum to 1 => bv contributes bv@Wo
    full = np.empty((B, N, E), dtype=np.float32)
    for b in range(B):
        acc = res.results[b * GROUPS]["out"].astype(np.float32).copy()
        for g in range(1, GROUPS):
            acc += res.results[b * GROUPS + g]["out"]
        full[b] = acc + bias_vec[None, :]
    return full


# revision 9
# speedup vs baseline: 1.1368x; 1.1368x over previous
"""Multi-head attention Trainium2 kernel (8 NeuronCores).

Problem: x[2,2048,1024] -> MHA(16 heads, d=64) -> out[2,2048,1024], fp32.

Sharding: 2-way data parallel on batch x 4-way tensor parallel on heads.
Core c handles batch c//4 and heads 4*(c%4) .. 4*(c%4)+3 (a 256-wide slice
of the Wq/Wk/Wv columns and Wo rows). Each core returns a partial output
[2048,1024]; the host sums the 4 TP partials per batch and adds the bias
terms (bo, and bv@Wo which is separable because softmax rows sum to 1;
bk drops out of softmax entirely since (q+bq)@bk is constant along keys).

On-core dataflow (all-bf16 operands, fp32 PSUM accumulation):
  xt = x[b].T (host-transposed, bf16)  [1024, 2048], DMA'd in 8 ec chunks
  V natural [2048, 256] via xt-stationary matmuls, ec-outer across 4 psum
    tiles so the chains consume each xt chunk as its DMA lands
  Q^T/K^T = W^T stationary over xt     [256, 2048]
  S^T[k,q] = K^T(d,k).T @ Q^T(d,q)     2 heads, d=64 each
  P = exp(S^T / 32)                    ScalarE only (1024-wide ACTs), bf16
  O'^T[d+1,q] = [V|ones].T @ P         ones column gives softmax denominators
  O^T = O'[0:64] * (1/denom)           DVE direct from PSUM, approx-recip
  out = O^T.T @ Wo_g                   [2048, 1024] partial, DMA'd out

The attention loop is software-pipelined so ScalarE (exp) stays saturated:
per kc the PE queue gets S(kc,h0) S(kc,h1) PV(kc-1,h1) PV(kc,h0); PV(kc,h1)
is deferred one iteration so it never head-of-line-blocks the next kc's S.
"""

import numpy as np

B = 2
N = 2048
E = 1024
HEADS = 16
D = 64
P = 128
NCORES = 8
GROUPS = 4            # TP groups
DG = E // GROUPS      # 256 cols per core
ECH = E // P          # 8 contraction chunks
NCH = N // P          # 16 sequence chunks
QS = 1024             # q span for softmax tiles
QB = 512              # matmul moving free dim

_CACHE = {}


def _build():
    import sys
    if "/opt/trn_rl_repo" not in sys.path:
        sys.path.insert(0, "/opt/trn_rl_repo")
    import concourse.tile as tile
    from concourse import bacc, mybir
    from concourse.bass import ts

    F32 = mybir.dt.float32
    BF16 = mybir.dt.bfloat16
    Exp = mybir.ActivationFunctionType.Exp

    nc = bacc.Bacc("TRN2", target_bir_lowering=False, debug=False, num_devices=NCORES)

    xt = nc.dram_tensor("xt", [E, N], BF16, kind="ExternalInput").ap()
    wq = nc.dram_tensor("wq", [E, DG], BF16, kind="ExternalInput").ap()
    wk = nc.dram_tensor("wk", [E, DG], BF16, kind="ExternalInput").ap()
    wv = nc.dram_tensor("wv", [E, DG], BF16, kind="ExternalInput").ap()
    wo = nc.dram_tensor("wo", [DG, E], BF16, kind="ExternalInput").ap()
    bq2 = nc.dram_tensor("bq2", [P, 2], F32, kind="ExternalInput").ap()
    out = nc.dram_tensor("out", [N, E], BF16, kind="ExternalOutput").ap()

    with tile.TileContext(nc) as tc:
        with tc.tile_pool(name="persist", bufs=1) as pers, \
             tc.tile_pool(name="pexp", bufs=12) as pexp_pool, \
             tc.tile_pool(name="small", bufs=2) as small, \
             tc.tile_pool(name="ostage", bufs=6) as ostage, \
             tc.tile_pool(name="ppmain", bufs=1, space="PSUM") as ppm, \
             tc.tile_pool(name="ppoacc", bufs=1, space="PSUM") as ppo:
            wq_sb = pers.tile([P, ECH, DG], BF16, tag="wq")
            wk_sb = pers.tile([P, ECH, DG], BF16, tag="wk")
            wv_sb = pers.tile([P, ECH, DG], BF16, tag="wv")
            wo_sb = pers.tile([P, 2, E], BF16, tag="wo")
            bq_sb = pers.tile([P, 2], F32, tag="bq")
            qT_p = [pers.tile([P, N], BF16, tag=f"qT{i}", name=f"qT{i}") for i in range(2)]
            kT_p = [pers.tile([P, N], BF16, tag=f"kT{i}", name=f"kT{i}") for i in range(2)]
            v_sb = pers.tile([P, NCH, GROUPS, 66], BF16, tag="v")
            oT_p = [pers.tile([P, N], BF16, tag=f"oT{i}", name=f"oT{i}") for i in range(2)]
            xt_sb = [pers.tile([P, N], BF16, tag=f"xt{ec}", name=f"xt{ec}")
                     for ec in range(ECH)]

            def psum_by_idx(i, name):
                pool, tag = [(ppm, "A"), (ppm, "B"), (ppo, "O0"), (ppo, "O1")][i % 4]
                return pool.tile([P, QS], F32, tag=tag, name=name)

            def qk_chain(pair, w_sb, dst, bias, qb):
                ps = psum_by_idx(qb, f"qkps{pair}{qb}")
                psl = ps[:, :QB]
                for ec in range(ECH):
                    nc.tensor.matmul(
                        psl,
                        w_sb[:, ec, ts(pair, P)],
                        xt_sb[ec][:, ts(qb, QB)],
                        start=(ec == 0), stop=(ec == ECH - 1),
                    )
                if bias:
                    nc.vector.tensor_add(
                        dst[:, ts(qb, QB)], psl,
                        bq_sb[:, pair, None].to_broadcast((P, QB)),
                    )
                else:
                    nc.vector.tensor_copy(dst[:, ts(qb, QB)], psl)

            def wo_chain(ncx, fb):
                ps = psum_by_idx(ncx * 2 + fb, f"wops{ncx}{fb}")
                psl = ps[:, :QB]
                for dc in range(2):
                    nc.tensor.matmul(
                        psl,
                        oT_p[dc][:, ts(ncx, P)],
                        wo_sb[:, dc, ts(fb, QB)],
                        start=(dc == 0), stop=(dc == 1),
                    )
                ot = ostage.tile([P, QB], BF16, tag="ot", name="ot")
                nc.vector.tensor_copy(ot, psl)
                nc.sync.dma_start(out[ts(ncx, P), ts(fb, QB)], ot)

            def emit_v():
                # ec-outer over 8 concurrent chains (one per PSUM bank: each
                # [P,1024] tile spans 2 banks, chains at cols 0 and 512 so a
                # chain's start=True bank-clear never hits a sibling chain)
                for g in range(2):
                    pss = [psum_by_idx(i, f"vps{g}{i}") for i in range(4)]
                    for ec in range(ECH):
                        for c in range(8):
                            ncx = 8 * g + c
                            nc.tensor.matmul(
                                pss[c // 2][:, (c % 2) * QB:(c % 2) * QB + DG],
                                xt_sb[ec][:, ts(ncx, P)],
                                wv_sb[:, ec, :],
                                start=(ec == 0), stop=(ec == ECH - 1),
                            )
                    for c in range(8):
                        ncx = 8 * g + c
                        nc.vector.tensor_copy(
                            v_sb[:, ncx, :, 0:64],
                            pss[c // 2][:, (c % 2) * QB:(c % 2) * QB + DG]
                            .rearrange("p (h d) -> p h d", d=D),
                        )

            def emit_attn(pair, qs):
                oaccs = [ppo.tile([65, QS], F32, tag=f"O{h}", name=f"oacc{h}")
                         for h in range(2)]
                pend = []  # deferred PV work: (kc, h, pe_tile)

                def pv(kc, h, pe):
                    for qb in range(QS // QB):
                        nc.tensor.matmul(
                            oaccs[h][:, ts(qb, QB)],
                            v_sb[:, kc, 2 * pair + h, 0:65],
                            pe[:, ts(qb, QB)],
                            start=(kc == 0), stop=(kc == NCH - 1),
                        )

                for kc in range(NCH):
                    pss = [ppm.tile([P, QS], F32, tag="AB"[h], name=f"spsum{h}")
                           for h in range(2)]
                    for h in range(2):
                        psl = slice(D * h, D * h + D)
                        for qb in range(QS // QB):
                            nc.tensor.matmul(
                                pss[h][:, ts(qb, QB)],
                                kT_p[pair][psl, ts(kc, P)],
                                qT_p[pair][psl, qs * QS + qb * QB:qs * QS + (qb + 1) * QB],
                                start=True, stop=True,
                            )
                    # drain 2 deferred PV matmul pairs (depth ~2 iterations:
                    # hides the previous emit's normalize from oacc reuse)
                    for _ in range(2):
                        if len(pend) > 4 - 2:
                            pv(*pend.pop(0))
                    for h in range(2):
                        pe = pexp_pool.tile([P, QS], BF16, tag="pexp", name="pexp")
                        nc.scalar.activation(pe, pss[h], Exp, scale=1.0 / 32.0)
                        pend.append((kc, h, pe))
                for w in pend:
                    pv(*w)
                for h in range(2):
                    dn = small.tile([1, QS], F32, tag=f"dn{h}", name="dn", bufs=1)
                    nc.vector.tensor_copy(dn, oaccs[h][64:65, :])
                    rc = small.tile([1, QS], F32, tag=f"rc{h}", name="rc", bufs=1)
                    nc.vector.reciprocal_approx_fast(rc, dn)
                    rbc = small.tile([64, QS], F32, tag="rbc", name="rbc")
                    nc.gpsimd.partition_broadcast(rbc, rc)
                    psl = slice(D * h, D * h + D)
                    nc.vector.tensor_mul(
                        oT_p[pair][psl, ts(qs, QS)],
                        oaccs[h][0:64, :],
                        rbc,
                    )

            nc.sync.dma_start(wv_sb, wv.rearrange("(c p) d -> p c d", p=P))
            xt_r = xt.rearrange("(c p) n -> p c n", p=P)
            for ec in range(4):
                nc.sync.dma_start(xt_sb[ec], xt_r[:, ec, :])
            nc.sync.dma_start(wk_sb, wk.rearrange("(c p) d -> p c d", p=P))
            nc.sync.dma_start(wq_sb, wq.rearrange("(c p) d -> p c d", p=P))
            for ec in range(4, ECH):
                nc.sync.dma_start(xt_sb[ec], xt_r[:, ec, :])
            nc.sync.dma_start(bq_sb, bq2)
            nc.sync.dma_start(wo_sb, wo.rearrange("(c p) f -> p c f", p=P))
            ones_f32 = pers.tile([P, 1], F32, tag="ones")
            nc.vector.memset(ones_f32, 1.0)
            nc.vector.tensor_copy(
                v_sb[:, :, :, 64:65],
                ones_f32[:, 0, None, None, None].to_broadcast((P, NCH, GROUPS, 1)),
            )
            emit_v()
            for qb in range(4):
                qk_chain(0, wk_sb, kT_p[0], False, qb)
            for qb in range(4):
                qk_chain(0, wq_sb, qT_p[0], True, qb)
            emit_attn(0, 0)
            emit_attn(0, 1)
            for qb in range(4):
                qk_chain(1, wk_sb, kT_p[1], False, qb)
            for qb in range(4):
                qk_chain(1, wq_sb, qT_p[1], True, qb)
            emit_attn(1, 0)
            emit_attn(1, 1)
            for ncx in range(NCH):
                for fb in range(2):
                    wo_chain(ncx, fb)

    nc.compile()
    return nc


def _get_nc():
    if "nc" not in _CACHE:
        _CACHE["nc"] = _build()
    return _CACHE["nc"]


def kernel(x, Wq, bq, Wk, bk, Wv, bv, Wo, bo, **run_kwargs):
    import sys
    if "/opt/trn_rl_repo" not in sys.path:
        sys.path.insert(0, "/opt/trn_rl_repo")
    import ml_dtypes
    from concourse.bass_utils import run_bass_kernel_spmd

    BF = ml_dtypes.bfloat16
    x = np.asarray(x, dtype=np.float32)
    Wq = np.asarray(Wq, dtype=np.float32)
    Wk = np.asarray(Wk, dtype=np.float32)
    Wv = np.asarray(Wv, dtype=np.float32)
    Wo = np.asarray(Wo, dtype=np.float32)
    bq = np.asarray(bq, dtype=np.float32)
    bv = np.asarray(bv, dtype=np.float32)
    bo = np.asarray(bo, dtype=np.float32)

    nc = _get_nc()

    in_maps = []
    xts = [np.ascontiguousarray(x[b].T).astype(BF) for b in range(B)]
    for c in range(NCORES):
        b, g = divmod(c, GROUPS)
        cols = slice(g * DG, (g + 1) * DG)
        in_maps.append({
            "xt": xts[b],
            "wq": np.ascontiguousarray(Wq[:, cols]).astype(BF),
            "wk": np.ascontiguousarray(Wk[:, cols]).astype(BF),
            "wv": np.ascontiguousarray(Wv[:, cols]).astype(BF),
            "wo": np.ascontiguousarray(Wo[cols, :]).astype(BF),
            "bq2": np.ascontiguousarray(bq[cols].reshape(2, P).T),
        })

    try:
        res = run_bass_kernel_spmd(nc, in_maps, core_ids=list(range(NCORES)), **run_kwargs)
    except Exception:
        # device may be wedged from a prior run; reset the accelerator once
        try:
            import ctypes
            lib = ctypes.CDLL("/opt/axon/libaxon_pjrt.so")
            lib.axon_reset.restype = ctypes.c_int
            lib.axon_reset()
        except Exception:
            pass
        res = run_bass_kernel_spmd(nc, in_maps, core_ids=list(range(NCORES)), **run_kwargs)
    if run_kwargs:
        _CACHE["last_results"] = res

    # gather: sum TP partials per batch, add separable bias terms
    bias_vec = bv @ Wo + bo  # softmax rows sum to 1 => bv contributes bv@Wo
    full = np.empty((B, N, E), dtype=np.float32)
    for b in range(B):
        acc = res.results[b * GROUPS]["out"].astype(np.float32).copy()
        for g in range(1, GROUPS):
            acc += res.results[b * GROUPS + g]["out"].astype(np.float32)
        full[b] = acc + bias_vec[None, :]
    return full


# revision 13
# speedup vs baseline: 1.2622x; 1.1103x over previous
"""Multi-head attention Trainium2 kernel (8 NeuronCores).

Problem: x[2,2048,1024] -> MHA(16 heads, d=64) -> out[2,2048,1024], fp32.

Sharding: 2-way data parallel on batch x 4-way tensor parallel on heads.
Core c handles batch c//4 and heads 4*(c%4) .. 4*(c%4)+3 (a 256-wide slice
of the Wq/Wk/Wv columns and Wo rows). Each core returns a partial output
[2048,1024]; the host sums the 4 TP partials per batch and adds the bias
terms (bo, and bv@Wo which is separable because softmax rows sum to 1;
bk drops out of softmax entirely since (q+bq)@bk is constant along keys).

On-core dataflow (all-bf16 operands, fp32 PSUM accumulation):
  xt = x[b].T (host-transposed, bf16)  [1024, 2048], DMA'd in 8 ec chunks
  V natural [2048, 256] via xt-stationary matmuls, ec-outer across 4 psum
    tiles so the chains consume each xt chunk as its DMA lands
  Q^T/K^T = W^T stationary over xt     [256, 2048]
  S^T[k,q] = K^T(d,k).T @ Q^T(d,q)     2 heads, d=64 each
  P = exp(S^T / 32)                    ScalarE only (1024-wide ACTs), bf16
  O'^T[d+1,q] = [V|ones].T @ P         ones column gives softmax denominators
  O^T = O'[0:64] * (1/denom)           DVE direct from PSUM, approx-recip
  out = O^T.T @ Wo_g                   [2048, 1024] partial, DMA'd out

The attention loop is software-pipelined so ScalarE (exp) stays saturated:
per kc the PE queue gets S(kc,h0) S(kc,h1) PV(kc-1,h1) PV(kc,h0); PV(kc,h1)
is deferred one iteration so it never head-of-line-blocks the next kc's S.
"""

import numpy as np

B = 2
N = 2048
E = 1024
HEADS = 16
D = 64
P = 128
NCORES = 8
GROUPS = 4            # TP groups
DG = E // GROUPS      # 256 cols per core
ECH = E // P          # 8 contraction chunks
NCH = N // P          # 16 sequence chunks
QS = 1024             # q span for softmax tiles
QB = 512              # matmul moving free dim

_CACHE = {}


def _build():
    import sys
    if "/opt/trn_rl_repo" not in sys.path:
        sys.path.insert(0, "/opt/trn_rl_repo")
    import concourse.tile as tile
    from concourse import bacc, mybir
    from concourse.bass import ts

    F32 = mybir.dt.float32
    BF16 = mybir.dt.bfloat16
    Exp = mybir.ActivationFunctionType.Exp

    nc = bacc.Bacc("TRN2", target_bir_lowering=False, debug=False, num_devices=NCORES)

    xt = nc.dram_tensor("xt", [E, N], BF16, kind="ExternalInput").ap()
    wq = nc.dram_tensor("wq", [E, DG], BF16, kind="ExternalInput").ap()
    wk = nc.dram_tensor("wk", [E, DG], BF16, kind="ExternalInput").ap()
    wv = nc.dram_tensor("wv", [E, DG], BF16, kind="ExternalInput").ap()
    wo = nc.dram_tensor("wo", [DG, E], BF16, kind="ExternalInput").ap()
    bq2 = nc.dram_tensor("bq2", [P, 2], F32, kind="ExternalInput").ap()
    out = nc.dram_tensor("out", [N, E], BF16, kind="ExternalOutput").ap()

    with tile.TileContext(nc) as tc:
        with tc.tile_pool(name="persist", bufs=1) as pers, \
             tc.tile_pool(name="pexp", bufs=12) as pexp_pool, \
             tc.tile_pool(name="small", bufs=2) as small, \
             tc.tile_pool(name="ostage", bufs=6) as ostage, \
             tc.tile_pool(name="ppmain", bufs=1, space="PSUM") as ppm, \
             tc.tile_pool(name="ppoacc", bufs=1, space="PSUM") as ppo, \
             tc.tile_pool(name="ppfill", bufs=1, space="PSUM") as ppf:
            wq_sb = pers.tile([P, ECH, DG], BF16, tag="wq")
            wk_sb = pers.tile([P, ECH, DG], BF16, tag="wk")
            wv_sb = pers.tile([P, ECH, DG], BF16, tag="wv")
            wo_sb = pers.tile([P, 2, E], BF16, tag="wo")
            bq_sb = pers.tile([P, 2], F32, tag="bq")
            qT_p = [pers.tile([P, N], BF16, tag=f"qT{i}", name=f"qT{i}") for i in range(2)]
            kT_p = [pers.tile([P, N], BF16, tag=f"kT{i}", name=f"kT{i}") for i in range(2)]
            v_sb = pers.tile([P, NCH, GROUPS, 66], BF16, tag="v")
            oT_p = [pers.tile([P, N], BF16, tag=f"oT{i}", name=f"oT{i}") for i in range(2)]
            xt_sb = [pers.tile([P, N], BF16, tag=f"xt{ec}", name=f"xt{ec}")
                     for ec in range(ECH)]

            def psum_by_idx(i, name):
                pool, tag = [(ppm, "A"), (ppm, "B"), (ppo, "O"), (ppf, "F")][i % 4]
                return pool.tile([P, QS], F32, tag=tag, name=name)

            def qk_chain(pair, w_sb, dst, bias, qb, pidx=None):
                ps = psum_by_idx(qb if pidx is None else pidx, f"qkps{pair}{qb}")
                psl = ps[:, :QB]
                for ec in range(ECH):
                    nc.tensor.matmul(
                        psl,
                        w_sb[:, ec, ts(pair, P)],
                        xt_sb[ec][:, ts(qb, QB)],
                        start=(ec == 0), stop=(ec == ECH - 1),
                    )
                if bias:
                    nc.vector.tensor_add(
                        dst[:, ts(qb, QB)], psl,
                        bq_sb[:, pair, None].to_broadcast((P, QB)),
                    )
                else:
                    nc.vector.tensor_copy(dst[:, ts(qb, QB)], psl)

            def wo_chain(ncx, fb, pidx=None):
                ps = psum_by_idx(ncx * 2 + fb if pidx is None else pidx,
                                 f"wops{ncx}{fb}")
                psl = ps[:, :QB]
                for dc in range(2):
                    nc.tensor.matmul(
                        psl,
                        oT_p[dc][:, ts(ncx, P)],
                        wo_sb[:, dc, ts(fb, QB)],
                        start=(dc == 0), stop=(dc == 1),
                    )
                ot = ostage.tile([P, QB], BF16, tag="ot", name="ot")
                nc.vector.tensor_copy(ot, psl)
                nc.sync.dma_start(out[ts(ncx, P), ts(fb, QB)], ot)

            def emit_v():
                # ec-outer over 8 concurrent chains (one per PSUM bank: each
                # [P,1024] tile spans 2 banks, chains at cols 0 and 512 so a
                # chain's start=True bank-clear never hits a sibling chain)
                for g in range(2):
                    pss = [psum_by_idx(i, f"vps{g}{i}") for i in range(4)]
                    for ec in range(ECH):
                        for c in range(8):
                            ncx = 8 * g + c
                            nc.tensor.matmul(
                                pss[c // 2][:, (c % 2) * QB:(c % 2) * QB + DG],
                                xt_sb[ec][:, ts(ncx, P)],
                                wv_sb[:, ec, :],
                                start=(ec == 0), stop=(ec == ECH - 1),
                            )
                    for c in range(8):
                        ncx = 8 * g + c
                        nc.vector.tensor_copy(
                            v_sb[:, ncx, :, 0:64],
                            pss[c // 2][:, (c % 2) * QB:(c % 2) * QB + DG]
                            .rearrange("p (h d) -> p h d", d=D),
                        )

            def emit_attn_h(pair, qs, h, fillers=()):
                # one head per pass: S ping-pongs two [P,QS] tiles (A/B by kc
                # parity), one oacc tile; the F psum bank pair stays free for
                # filler chains (projections / Wo) to absorb PE idle while
                # ScalarE chews the exps.
                fillers = list(fillers)
                spacing = max(2, NCH // (len(fillers) + 1)) if fillers else NCH
                oacc = ppo.tile([65, QS], F32, tag="O", name="oacc")
                hh = 2 * pair + h
                psl = slice(D * h, D * h + D)
                pend = []  # deferred PV work: (kc, pe_tile)

                def pv(kc, pe):
                    for qb in range(QS // QB):
                        nc.tensor.matmul(
                            oacc[:, ts(qb, QB)],
                            v_sb[:, kc, hh, 0:65],
                            pe[:, ts(qb, QB)],
                            start=(kc == 0), stop=(kc == NCH - 1),
                        )

                for kc in range(NCH):
                    if fillers and kc >= 1 and kc % spacing == 0:
                        fillers.pop(0)()
                    ps = ppm.tile([P, QS], F32, tag="AB"[kc % 2], name="spsum")
                    for qb in range(QS // QB):
                        nc.tensor.matmul(
                            ps[:, ts(qb, QB)],
                            kT_p[pair][psl, ts(kc, P)],
                            qT_p[pair][psl, qs * QS + qb * QB:qs * QS + (qb + 1) * QB],
                            start=True, stop=True,
                        )
                    if len(pend) >= 3:
                        pv(*pend.pop(0))
                    pe = pexp_pool.tile([P, QS], BF16, tag="pexp", name="pexp")
                    nc.scalar.activation(pe, ps, Exp, scale=1.0 / 32.0)
                    pend.append((kc, pe))
                for f in fillers:
                    f()
                for w in pend:
                    pv(*w)
                dn = small.tile([1, QS], F32, tag="dn", name="dn", bufs=2)
                nc.vector.tensor_copy(dn, oacc[64:65, :])
                rc = small.tile([1, QS], F32, tag="rc", name="rc", bufs=2)
                nc.vector.reciprocal_approx_fast(rc, dn)
                rbc = small.tile([64, QS], F32, tag="rbc", name="rbc")
                nc.gpsimd.partition_broadcast(rbc, rc)
                nc.vector.tensor_mul(
                    oT_p[pair][psl, ts(qs, QS)],
                    oacc[0:64, :],
                    rbc,
                )

            nc.sync.dma_start(wv_sb, wv.rearrange("(c p) d -> p c d", p=P))
            xt_r = xt.rearrange("(c p) n -> p c n", p=P)
            for ec in range(4):
                nc.sync.dma_start(xt_sb[ec], xt_r[:, ec, :])
            nc.sync.dma_start(wk_sb, wk.rearrange("(c p) d -> p c d", p=P))
            nc.sync.dma_start(wq_sb, wq.rearrange("(c p) d -> p c d", p=P))
            for ec in range(4, ECH):
                nc.sync.dma_start(xt_sb[ec], xt_r[:, ec, :])
            nc.sync.dma_start(bq_sb, bq2)
            nc.sync.dma_start(wo_sb, wo.rearrange("(c p) f -> p c f", p=P))
            ones_f32 = pers.tile([P, 1], F32, tag="ones")
            nc.vector.memset(ones_f32, 1.0)
            nc.vector.tensor_copy(
                v_sb[:, :, :, 64:65],
                ones_f32[:, 0, None, None, None].to_broadcast((P, NCH, GROUPS, 1)),
            )
            emit_v()
            for qb in range(4):
                qk_chain(0, wk_sb, kT_p[0], False, qb)
            for qb in range(4):
                qk_chain(0, wq_sb, qT_p[0], True, qb)

            def f_qk(pair, w, dst, bias, qb):
                return lambda: qk_chain(pair, w, dst, bias, qb, pidx=3)

            def f_wo(ncx, fb):
                return lambda: wo_chain(ncx, fb, pidx=3)

            # pair-1 K/Q projections ride as fillers inside pair-0 attention;
            # qs0 Wo chains (and their out-DMAs) ride inside attn(1,1)
            emit_attn_h(0, 0, 0, [f_qk(1, wk_sb, kT_p[1], False, 0),
                                  f_qk(1, wq_sb, qT_p[1], True, 0)])
            emit_attn_h(0, 0, 1, [f_qk(1, wk_sb, kT_p[1], False, 1),
                                  f_qk(1, wq_sb, qT_p[1], True, 1)])
            emit_attn_h(0, 1, 0, [f_qk(1, wk_sb, kT_p[1], False, 2),
                                  f_qk(1, wq_sb, qT_p[1], True, 2)])
            emit_attn_h(0, 1, 1, [f_qk(1, wk_sb, kT_p[1], False, 3),
                                  f_qk(1, wq_sb, qT_p[1], True, 3)])
            emit_attn_h(1, 0, 0)
            emit_attn_h(1, 0, 1)
            emit_attn_h(1, 1, 0, [f_wo(ncx, 0) for ncx in range(8)])
            emit_attn_h(1, 1, 1, [f_wo(ncx, 1) for ncx in range(8)])
            for ncx in range(8, NCH):
                for fb in range(2):
                    wo_chain(ncx, fb)

    nc.compile()
    return nc


def _get_nc():
    if "nc" not in _CACHE:
        _CACHE["nc"] = _build()
    return _CACHE["nc"]


def kernel(x, Wq, bq, Wk, bk, Wv, bv, Wo, bo, **run_kwargs):
    import sys
    if "/opt/trn_rl_repo" not in sys.path:
        sys.path.insert(0, "/opt/trn_rl_repo")
    import ml_dtypes
    from concourse.bass_utils import run_bass_kernel_spmd

    BF = ml_dtypes.bfloat16
    x = np.asarray(x, dtype=np.float32)
    Wq = np.asarray(Wq, dtype=np.float32)
    Wk = np.asarray(Wk, dtype=np.float32)
    Wv = np.asarray(Wv, dtype=np.float32)
    Wo = np.asarray(Wo, dtype=np.float32)
    bq = np.asarray(bq, dtype=np.float32)
    bv = np.asarray(bv, dtype=np.float32)
    bo = np.asarray(bo, dtype=np.float32)

    nc = _get_nc()

    in_maps = []
    xts = [np.ascontiguousarray(x[b].T).astype(BF) for b in range(B)]
    for c in range(NCORES):
        b, g = divmod(c, GROUPS)
        cols = slice(g * DG, (g + 1) * DG)
        in_maps.append({
            "xt": xts[b],
            "wq": np.ascontiguousarray(Wq[:, cols]).astype(BF),
            "wk": np.ascontiguousarray(Wk[:, cols]).astype(BF),
            "wv": np.ascontiguousarray(Wv[:, cols]).astype(BF),
            "wo": np.ascontiguousarray(Wo[cols, :]).astype(BF),
            "bq2": np.ascontiguousarray(bq[cols].reshape(2, P).T),
        })

    try:
        res = run_bass_kernel_spmd(nc, in_maps, core_ids=list(range(NCORES)), **run_kwargs)
    except Exception:
        # device may be wedged from a prior run; reset the accelerator once
        try:
            import ctypes
            lib = ctypes.CDLL("/opt/axon/libaxon_pjrt.so")
            lib.axon_reset.restype = ctypes.c_int
            lib.axon_reset()
        except Exception:
            pass
        res = run_bass_kernel_spmd(nc, in_maps, core_ids=list(range(NCORES)), **run_kwargs)
    if run_kwargs:
        _CACHE["last_results"] = res

    # gather: sum TP partials per batch, add separable bias terms
    bias_vec = bv @ Wo + bo  # softmax rows sum to 1 => bv contributes bv@Wo
    full = np.empty((B, N, E), dtype=np.float32)
    for b in range(B):
        acc = res.results[b * GROUPS]["out"].astype(np.float32).copy()
        for g in range(1, GROUPS):
            acc += res.results[b * GROUPS + g]["out"].astype(np.float32)
        full[b] = acc + bias_vec[None, :]
    return full
